# revision 19
# baseline (speedup 1.0000x reference)
"""Bass/Trainium2 kernel for nn_CustomLinearRNN.

Reference recurrence (T=4096, K=64, H=128, B=128):
    known_t = A1 @ x_t + A2 @ h_{t-1}
    h_t     = A3 @ x_t + A4 @ h_{t-1}
returns (known_seq [T,K,B], hidden_seq [T,H,B]).

Strategy: pure data-parallel over batch (B=128 -> 16 per core, 8 cores).
Per core the sequential scan is restructured into batched matmuls (fp32r):
  - time split into chunks of Tc=256; each chunk into Q=32 fine blocks of
    m=8 steps; columns laid out r-major (r = s%8) so every lagged matmul
    hits contiguous 512-column slices.
  - V2 = depth-4 causal conv of x with (A4^d A3), d=0..3 (masked at fine
    block boundaries), accumulated in PSUM.
  - fine level 3: V3[s] = V2[s] + A4^4 V2[s-4] (folded into broadcast).
  - block-end sums E_q = V3 at r=7; a seeded radix-4 Hillis-Steele scan
    over the 65-entry pair sequence [h_in, E_0..E_63] with powers of A4^8
    yields S_{q-1} = state entering each fine block + the pair-exit state.
  - broadcast: H[s] = V2[s] + (r>=4: A4^4 V2[s-4]) + A4^{r+1} S_{q-1}.
  - known[s] = A1 x[s] + A2 h[s-1]; h[s-1] = (r-1)-slice of H, or S_{q-1}
    for r=0; r and r+4 slices share one PSUM tile via partition packing.
All matrix powers/products precomputed on host in float64.
"""

import numpy as np

T, K, H, B = 4096, 64, 128, 128
NCORES = 8
BL = B // NCORES            # 16 batch per core
TC = 256                    # chunk length (timesteps)
NCH = T // TC               # 16 chunks
M = 8                       # fine block
Q = TC // M                 # 32 fine blocks per chunk
RS = Q * BL                 # 512 = columns per r-slice
CC = TC * BL                # 4096 = columns per chunk
NCOL = T * BL               # 65536 columns per core
NE = 2 * Q + 1              # 65 scan entries per chunk-pair
EC = NE * BL                # 1040 columns in the pair scan buffer
E8 = [1, 2, 3, 4, 8, 12, 16, 32, 48, 64]   # powers of A4^8 for the scan

_cache = {}


def _build_program():
    import concourse.tile as tile
    from concourse import bacc, mybir

    nc = bacc.Bacc("TRN2", target_bir_lowering=False, debug=False,
                   num_devices=NCORES)
    f32 = mybir.dt.float32
    f32r = mybir.dt.float32r

    xin = nc.dram_tensor("xin", [K, NCOL], f32r, kind="ExternalInput").ap()
    h0in = nc.dram_tensor("h0in", [H, BL], f32r, kind="ExternalInput").ap()
    wconv = nc.dram_tensor("wconv", [K, 4 * H], f32r,
                           kind="ExternalInput").ap()
    p4in = nc.dram_tensor("p4in", [H, H], f32r, kind="ExternalInput").ap()
    lagin = nc.dram_tensor("lagin", [H, 8 * H], f32r,
                           kind="ExternalInput").ap()
    qin = nc.dram_tensor("qin", [H, len(E8) * H], f32r,
                         kind="ExternalInput").ap()
    a1in = nc.dram_tensor("a1in", [K, K], f32r, kind="ExternalInput").ap()
    a2in = nc.dram_tensor("a2in", [H, K], f32r, kind="ExternalInput").ap()
    hid = nc.dram_tensor("hid", [H, NCOL], f32, kind="ExternalOutput").ap()
    kn = nc.dram_tensor("kn", [2 * K, NCOL // 2], f32,
                        kind="ExternalOutput").ap()

    with tile.TileContext(nc) as tc:
        from contextlib import ExitStack
        with ExitStack() as ctx:
            wp = ctx.enter_context(tc.tile_pool(name="weights", bufs=1))
            xp = ctx.enter_context(tc.tile_pool(name="xtiles", bufs=3))
            v2p = ctx.enter_context(tc.tile_pool(name="v2", bufs=4))
            hp = ctx.enter_context(tc.tile_pool(name="ht", bufs=3))
            knp = ctx.enter_context(tc.tile_pool(name="kn", bufs=2))
            ep = ctx.enter_context(tc.tile_pool(name="escan", bufs=2))
            pp = ctx.enter_context(
                tc.tile_pool(name="psum", bufs=3, space="PSUM"))

            w_all = wp.tile([2 * K, 4 * H], f32r, tag="w_all")
            nc.sync.dma_start(w_all[0:K, :], wconv)
            nc.sync.dma_start(w_all[K:2 * K, :], wconv)
            p4 = wp.tile([H, H], f32r, tag="p4")
            nc.sync.dma_start(p4[:], p4in)
            lag = wp.tile([H, 8 * H], f32r, tag="lag")
            nc.sync.dma_start(lag[:], lagin)
            qw = wp.tile([H, len(E8) * H], f32r, tag="qw")
            nc.sync.dma_start(qw[:], qin)
            a1 = wp.tile([2 * K, K], f32r, tag="a1")
            nc.sync.dma_start(a1[0:K, :], a1in)
            nc.sync.dma_start(a1[K:2 * K, :], a1in)
            a2 = wp.tile([H, K], f32r, tag="a2")
            nc.sync.dma_start(a2[:], a2in)
            h0 = wp.tile([H, BL], f32r, tag="h0")
            nc.sync.dma_start(h0[:], h0in)

            def G(e):  # lhsT of (A4^8)^e
                j = E8.index(e)
                return qw[:, j * H:(j + 1) * H]

            NP = NCH // 2
            xts = {}
            v2s = {}          # chunk index -> V2 tile

            def emit_xt(pair):
                xt = xp.tile([2 * K, CC], f32r, tag="xt",
                             name=f"xt_{pair}")
                xts[pair] = xt
                npc = 4 if pair == 0 else 2
                for par in range(2):
                    cbase = (2 * pair + par) * CC
                    for piece in range(npc):
                        sl = slice(piece * CC // npc,
                                   (piece + 1) * CC // npc)
                        nc.sync.dma_start(
                            xt[par * K:(par + 1) * K, sl],
                            xin[:, cbase + piece * CC // npc:
                                cbase + (piece + 1) * CC // npc])

            def conv_pieces(pair):
                """Yield conv emission pieces for both chunks of a pair."""
                for par in range(2):
                    c = 2 * pair + par
                    xv = xts[pair][par * K:(par + 1) * K, :]
                    v2 = v2p.tile([H, CC], f32r, tag="v2", name=f"v2_{c}")
                    v2s[c] = v2
                    cps = [pp.tile([H, RS], f32, tag="ps", bufs=6,
                                   name=f"cps{c}_{i}") for i in range(8)]

                    def piece(d, par=par, c=c, xv=xv, v2=v2, cps=cps):
                        wd = w_all[par * K:(par + 1) * K,
                                   d * H:(d + 1) * H]
                        for r in range(d, 8):
                            nc.tensor.matmul(
                                cps[r][:],
                                wd, xv[:, (r - d) * RS:(r - d + 1) * RS],
                                start=(d == 0), stop=(d == min(r, 3)))
                        lo, hi = (d, d + 1) if d < 3 else (3, 8)
                        for r in range(lo, hi):
                            nc.scalar.copy(v2[:, r * RS:(r + 1) * RS],
                                           cps[r][:])
                    yield from (lambda d=d: piece(d) for d in range(4))

            def emit_scan(pair, prev_last, fillers):
                """Seeded radix-4 scan; calls next(fillers) between rounds."""
                eb = ep.tile([H, EC], f32r, tag="eb", name=f"eb_{pair}")
                nc.vector.tensor_copy(eb[:, 0:BL], prev_last[:, 0:BL])
                for par in range(2):
                    xps = pp.tile([H, RS], f32, tag="eps", bufs=2,
                                  name=f"xps{pair}_{par}")
                    nc.tensor.matmul(xps[:], p4[:],
                                     v2s[2 * pair + par][:, 3 * RS:4 * RS],
                                     start=True, stop=True)
                    nc.vector.tensor_add(
                        eb[:, (1 + par * Q) * BL:(1 + (par + 1) * Q) * BL],
                        v2s[2 * pair + par][:, 7 * RS:8 * RS], xps[:])
                for base in (1, 4, 16):
                    for f in (next(fillers, None), next(fillers, None)):
                        if f is not None:
                            f()
                    a_lags = [v for v in (base, 2 * base, 3 * base)
                              if v * BL < RS + BL]
                    b_lags = [base, 2 * base, 3 * base]
                    if base == 16:
                        b_lags.append(64)
                    pa = pp.tile([H, RS], f32, tag="eps", bufs=2,
                                 name=f"pa{pair}_{base}")
                    pb = pp.tile([H, RS], f32, tag="eps", bufs=2,
                                 name=f"pb{pair}_{base}")
                    na = RS + BL - base * BL   # cols in part a
                    for i, v in enumerate(a_lags):
                        nc.tensor.matmul(
                            pa[:, (v - base) * BL:na],
                            G(v), eb[:, 0:RS + BL - v * BL],
                            start=(i == 0), stop=(i == len(a_lags) - 1))
                    for i, v in enumerate(b_lags):
                        lo = max(RS + BL, v * BL)
                        nc.tensor.matmul(
                            pb[:, lo - (RS + BL):RS],
                            G(v), eb[:, lo - v * BL:EC - v * BL],
                            start=(i == 0), stop=(i == len(b_lags) - 1))
                    nc.vector.tensor_add(eb[:, base * BL:RS + BL],
                                         eb[:, base * BL:RS + BL],
                                         pa[:, 0:na])
                    nc.vector.tensor_add(eb[:, RS + BL:EC],
                                         eb[:, RS + BL:EC], pb[:])
                for f in fillers:
                    f()
                return eb

            def emit_b(pair, eb):
                for par in range(2):
                    c = 2 * pair + par
                    xv = xts[pair][par * K:(par + 1) * K, :]
                    v2 = v2s[c]
                    cmat = eb[:, par * RS:(par + 1) * RS]
                    ht = hp.tile([H, CC], f32r, tag="ht", name=f"ht_{c}")
                    bps = [pp.tile([H, RS], f32, tag="ps", bufs=6,
                                   name=f"bps{c}_{i}") for i in range(8)]
                    for r in range(4, 8):
                        nc.tensor.matmul(
                            bps[r][:],
                            p4[:], v2[:, (r - 4) * RS:(r - 3) * RS],
                            start=True, stop=False)
                    for r in range(8):
                        nc.tensor.matmul(
                            bps[r][:],
                            lag[:, r * H:(r + 1) * H], cmat,
                            start=(r < 4), stop=True)
                    for r in range(8):
                        nc.vector.tensor_add(
                            ht[:, r * RS:(r + 1) * RS],
                            v2[:, r * RS:(r + 1) * RS], bps[r][:])
                    for piece in range(2):
                        sl = slice(piece * CC // 2, (piece + 1) * CC // 2)
                        nc.sync.dma_start(
                            hid[:, c * CC + piece * CC // 2:
                                c * CC + (piece + 1) * CC // 2],
                            ht[:, sl].bitcast(f32))

                    knt = knp.tile([2 * K, 4 * RS], f32, tag="knt",
                                   name=f"knt_{c}")
                    kps = [pp.tile([H, RS], f32, tag="ps", bufs=6,
                                   name=f"kps{c}_{i}") for i in range(8)]
                    a1v = a1[par * K:(par + 1) * K, :]
                    for r in range(8):
                        nc.tensor.matmul(kps[r][0:K, :], a1v,
                                         xv[:, r * RS:(r + 1) * RS],
                                         start=True, stop=False)
                    for r in range(8):
                        hpv = (ht[:, (r - 1) * RS:r * RS] if r >= 1
                               else cmat)
                        nc.tensor.matmul(kps[r][0:K, :], a2[:], hpv,
                                         start=False, stop=True)
                    for r in range(8):
                        # rows: r<4 -> knt[0:K] col-block r; r>=4 ->
                        # knt[K:2K] col-block r-4 (ACT/DVE place across
                        # partition halves; alternate engines to balance)
                        dst = (knt[0:K, r * RS:(r + 1) * RS] if r < 4 else
                               knt[K:2 * K, (r - 4) * RS:(r - 3) * RS])
                        if r % 2 == 0:
                            nc.scalar.copy(dst, kps[r][0:K, :])
                        else:
                            nc.vector.tensor_copy(dst, kps[r][0:K, :])
                    for piece in range(2):
                        nc.sync.dma_start(
                            kn[:, c * 4 * RS + piece * 2 * RS:
                                c * 4 * RS + (piece + 1) * 2 * RS],
                            knt[:, piece * 2 * RS:(piece + 1) * 2 * RS])

            # software-pipelined emission: scan(i) interleaved with conv(i+1)
            prev_last = h0
            emit_xt(0)
            for f in conv_pieces(0):
                f()
            for i in range(NP):
                if i + 1 < NP:
                    emit_xt(i + 1)
                    fillers = conv_pieces(i + 1)
                else:
                    fillers = iter(())
                eb = emit_scan(i, prev_last, fillers)
                prev_last = eb[:, 64 * BL:EC]
                emit_b(i, eb)

    nc.compile()
    return nc


def _get_program():
    if "nc" not in _cache:
        _cache["nc"] = _build_program()
    return _cache["nc"]


def _host_weights(A1, A2, A3, A4):
    A1d, A2d, A3d, A4d = (np.asarray(a, np.float64) for a in (A1, A2, A3, A4))
    pw = [np.eye(H)]
    for _ in range(8):
        pw.append(pw[-1] @ A4d)
    wconv = np.concatenate(
        [(pw[d] @ A3d).T for d in range(4)], axis=1).astype(np.float32)
    p4 = pw[4].T.astype(np.float32)
    lag = np.concatenate(
        [pw[r + 1].T for r in range(8)], axis=1).astype(np.float32)
    g8 = pw[8]
    qmats = []
    for e in E8:
        qmats.append(np.linalg.matrix_power(g8, e).T.astype(np.float32))
    qin = np.concatenate(qmats, axis=1)
    return (np.ascontiguousarray(wconv), np.ascontiguousarray(p4),
            np.ascontiguousarray(lag), np.ascontiguousarray(qin),
            np.ascontiguousarray(A1d.T.astype(np.float32)),
            np.ascontiguousarray(A2d.T.astype(np.float32)))


def _in_maps(x, h0, weights):
    wconv, p4, lag, qin, a1t, a2t = weights
    maps = []
    for core in range(NCORES):
        xc = x[:, :, core * BL:(core + 1) * BL]          # [T, K, BL]
        xr = xc.reshape(NCH, Q, M, K, BL)                # [c, q, r, k, b]
        xdev = np.ascontiguousarray(
            xr.transpose(3, 0, 2, 1, 4).reshape(K, NCOL))  # [k,(c,r,q,b)]
        maps.append({
            "xin": xdev,
            "h0in": np.ascontiguousarray(h0[:, core * BL:(core + 1) * BL]),
            "wconv": wconv, "p4in": p4, "lagin": lag, "qin": qin,
            "a1in": a1t, "a2in": a2t,
        })
    return maps


def kernel(x, init_hidden_states, A1, A2, A3, A4):
    from concourse.bass_utils import run_bass_kernel_spmd

    x = np.asarray(x, np.float32)
    h0 = np.asarray(init_hidden_states, np.float32)
    weights = _host_weights(A1, A2, A3, A4)

    nc = _get_program()
    results = run_bass_kernel_spmd(
        nc, _in_maps(x, h0, weights), core_ids=list(range(NCORES))).results

    known = np.empty((T, K, B), np.float32)
    hidden = np.empty((T, H, B), np.float32)
    for core in range(NCORES):
        bs = slice(core * BL, (core + 1) * BL)
        hd = results[core]["hid"].reshape(H, NCH, M, Q, BL)
        hidden[:, :, bs] = hd.transpose(1, 3, 2, 0, 4).reshape(T, H, BL)
        # kn rows: p<64 -> k=p, r=j; p>=64 -> k=p-64, r=j+4
        kd = results[core]["kn"].reshape(2, K, NCH, 4, Q, BL)
        kd2 = kd.transpose(2, 4, 0, 3, 1, 5)  # [c, q, half, j, k, b]
        kd2 = kd2.reshape(NCH, Q, M, K, BL)   # [c, q, r, k, b], r=half*4+j
        known[:, :, bs] = kd2.reshape(T, K, BL)
    return known, hidden


# revision 24
# speedup vs baseline: 1.0424x; 1.0424x over previous
"""Bass/Trainium2 kernel for nn_CustomLinearRNN.

Reference recurrence (T=4096, K=64, H=128, B=128):
    known_t = A1 @ x_t + A2 @ h_{t-1}
    h_t     = A3 @ x_t + A4 @ h_{t-1}
returns (known_seq [T,K,B], hidden_seq [T,H,B]).

Strategy: pure data-parallel over batch (B=128 -> 16 per core, 8 cores).
Per core the sequential scan is restructured into batched matmuls (fp32r):
  - time split into chunks of Tc=256; each chunk into Q=32 fine blocks of
    m=8 steps; columns laid out r-major (r = s%8) so every lagged matmul
    hits contiguous 512-column slices.
  - V2 = depth-4 causal conv of x with (A4^d A3), d=0..3 (masked at fine
    block boundaries), accumulated in PSUM.
  - fine level 3: V3[s] = V2[s] + A4^4 V2[s-4] (folded into broadcast).
  - block-end sums E_q = V3 at r=7; a seeded radix-4 Hillis-Steele scan
    over the 65-entry pair sequence [h_in, E_0..E_63] with powers of A4^8
    yields S_{q-1} = state entering each fine block + the pair-exit state.
  - broadcast: H[s] = V2[s] + (r>=4: A4^4 V2[s-4]) + A4^{r+1} S_{q-1}.
  - known[s] = A1 x[s] + A2 h[s-1]; h[s-1] = (r-1)-slice of H, or S_{q-1}
    for r=0; r and r+4 slices share one PSUM tile via partition packing.
All matrix powers/products precomputed on host in float64.
"""

import numpy as np

T, K, H, B = 4096, 64, 128, 128
NCORES = 8
BL = B // NCORES            # 16 batch per core
TC = 256                    # chunk length (timesteps)
NCH = T // TC               # 16 chunks
M = 8                       # fine block
Q = TC // M                 # 32 fine blocks per chunk
RS = Q * BL                 # 512 = columns per r-slice
CC = TC * BL                # 4096 = columns per chunk
NCOL = T * BL               # 65536 columns per core
NE = 2 * Q + 1              # 65 scan entries per chunk-pair
EC = NE * BL                # 1040 columns in the pair scan buffer
E8 = [1, 2, 3, 4, 8, 12, 16, 32, 48, 64]   # powers of A4^8 for the scan

_cache = {}


def _build_program():
    import concourse.tile as tile
    from concourse import bacc, mybir

    nc = bacc.Bacc("TRN2", target_bir_lowering=False, debug=False,
                   num_devices=NCORES)
    f32 = mybir.dt.float32
    f32r = mybir.dt.float32r

    xin = nc.dram_tensor("xin", [K, NCOL], f32r, kind="ExternalInput").ap()
    h0in = nc.dram_tensor("h0in", [H, BL], f32r, kind="ExternalInput").ap()
    wconv = nc.dram_tensor("wconv", [K, 4 * H], f32r,
                           kind="ExternalInput").ap()
    p4in = nc.dram_tensor("p4in", [H, H], f32r, kind="ExternalInput").ap()
    lagin = nc.dram_tensor("lagin", [H, 8 * H], f32r,
                           kind="ExternalInput").ap()
    qin = nc.dram_tensor("qin", [H, len(E8) * H], f32r,
                         kind="ExternalInput").ap()
    a1in = nc.dram_tensor("a1in", [K, K], f32r, kind="ExternalInput").ap()
    a2in = nc.dram_tensor("a2in", [H, K], f32r, kind="ExternalInput").ap()
    hid = nc.dram_tensor("hid", [H, NCOL], f32, kind="ExternalOutput").ap()
    kn = nc.dram_tensor("kn", [2 * K, NCOL // 2], f32,
                        kind="ExternalOutput").ap()

    with tile.TileContext(nc) as tc:
        from contextlib import ExitStack
        with ExitStack() as ctx:
            wp = ctx.enter_context(tc.tile_pool(name="weights", bufs=1))
            xp = ctx.enter_context(tc.tile_pool(name="xtiles", bufs=3))
            v2p = ctx.enter_context(tc.tile_pool(name="v2", bufs=4))
            hp = ctx.enter_context(tc.tile_pool(name="ht", bufs=3))
            knp = ctx.enter_context(tc.tile_pool(name="kn", bufs=2))
            ep = ctx.enter_context(tc.tile_pool(name="escan", bufs=2))
            pp = ctx.enter_context(
                tc.tile_pool(name="psum", bufs=3, space="PSUM"))

            w_all = wp.tile([2 * K, 4 * H], f32r, tag="w_all")
            nc.sync.dma_start(w_all[0:K, :], wconv)
            nc.sync.dma_start(w_all[K:2 * K, :], wconv)
            p4 = wp.tile([H, H], f32r, tag="p4")
            nc.sync.dma_start(p4[:], p4in)
            lag = wp.tile([H, 8 * H], f32r, tag="lag")
            nc.sync.dma_start(lag[:], lagin)
            qw = wp.tile([H, len(E8) * H], f32r, tag="qw")
            nc.sync.dma_start(qw[:], qin)
            a1 = wp.tile([2 * K, K], f32r, tag="a1")
            nc.sync.dma_start(a1[0:K, :], a1in)
            nc.sync.dma_start(a1[K:2 * K, :], a1in)
            a2 = wp.tile([H, K], f32r, tag="a2")
            nc.sync.dma_start(a2[:], a2in)
            h0 = wp.tile([H, BL], f32r, tag="h0")
            nc.sync.dma_start(h0[:], h0in)

            def G(e):  # lhsT of (A4^8)^e
                j = E8.index(e)
                return qw[:, j * H:(j + 1) * H]

            NP = NCH // 2
            xts = {}
            v2s = {}          # chunk index -> V2 tile

            def emit_xt(pair):
                xt = xp.tile([2 * K, CC], f32r, tag="xt",
                             name=f"xt_{pair}")
                xts[pair] = xt
                npc = 8 if pair == 0 else 2
                for par in range(2):
                    cbase = (2 * pair + par) * CC
                    for piece in range(npc):
                        sl = slice(piece * CC // npc,
                                   (piece + 1) * CC // npc)
                        nc.sync.dma_start(
                            xt[par * K:(par + 1) * K, sl],
                            xin[:, cbase + piece * CC // npc:
                                cbase + (piece + 1) * CC // npc])

            def conv_pieces(pair):
                """Yield conv emission pieces for both chunks of a pair."""
                for par in range(2):
                    c = 2 * pair + par
                    xv = xts[pair][par * K:(par + 1) * K, :]
                    v2 = v2p.tile([H, CC], f32r, tag="v2", name=f"v2_{c}")
                    v2s[c] = v2
                    cps = [pp.tile([H, RS], f32, tag="ps", bufs=6,
                                   name=f"cps{c}_{i}") for i in range(8)]

                    def piece(d, par=par, c=c, xv=xv, v2=v2, cps=cps):
                        wd = w_all[par * K:(par + 1) * K,
                                   d * H:(d + 1) * H]
                        for r in range(d, 8):
                            nc.tensor.matmul(
                                cps[r][:],
                                wd, xv[:, (r - d) * RS:(r - d + 1) * RS],
                                start=(d == 0), stop=(d == min(r, 3)))
                        lo, hi = (d, d + 1) if d < 3 else (3, 8)
                        for r in range(lo, hi):
                            nc.scalar.copy(v2[:, r * RS:(r + 1) * RS],
                                           cps[r][:])
                    yield from (lambda d=d: piece(d) for d in range(4))

            def emit_scan(pair, prev_last, fillers):
                """Seeded radix-4 scan; calls next(fillers) between rounds."""
                eb = ep.tile([H, EC], f32r, tag="eb", name=f"eb_{pair}")
                nc.vector.tensor_copy(eb[:, 0:BL], prev_last[:, 0:BL])
                for par in range(2):
                    xps = pp.tile([H, RS], f32, tag="eps", bufs=2,
                                  name=f"xps{pair}_{par}")
                    nc.tensor.matmul(xps[:], p4[:],
                                     v2s[2 * pair + par][:, 3 * RS:4 * RS],
                                     start=True, stop=True)
                    nc.vector.tensor_add(
                        eb[:, (1 + par * Q) * BL:(1 + (par + 1) * Q) * BL],
                        v2s[2 * pair + par][:, 7 * RS:8 * RS], xps[:])
                for base in (1, 4, 16):
                    for f in (next(fillers, None), next(fillers, None)):
                        if f is not None:
                            f()
                    a_lags = [v for v in (base, 2 * base, 3 * base)
                              if v * BL < RS + BL]
                    b_lags = [base, 2 * base, 3 * base]
                    if base == 16:
                        b_lags.append(64)
                    pa = pp.tile([H, RS], f32, tag="eps", bufs=2,
                                 name=f"pa{pair}_{base}")
                    pb = pp.tile([H, RS], f32, tag="eps", bufs=2,
                                 name=f"pb{pair}_{base}")
                    na = RS + BL - base * BL   # cols in part a
                    for i, v in enumerate(a_lags):
                        nc.tensor.matmul(
                            pa[:, (v - base) * BL:na],
                            G(v), eb[:, 0:RS + BL - v * BL],
                            start=(i == 0), stop=(i == len(a_lags) - 1))
                    for i, v in enumerate(b_lags):
                        lo = max(RS + BL, v * BL)
                        nc.tensor.matmul(
                            pb[:, lo - (RS + BL):RS],
                            G(v), eb[:, lo - v * BL:EC - v * BL],
                            start=(i == 0), stop=(i == len(b_lags) - 1))
                    nc.vector.tensor_add(eb[:, base * BL:RS + BL],
                                         eb[:, base * BL:RS + BL],
                                         pa[:, 0:na])
                    nc.vector.tensor_add(eb[:, RS + BL:EC],
                                         eb[:, RS + BL:EC], pb[:])
                for f in fillers:
                    f()
                return eb

            def emit_b(pair, eb):
                for par in range(2):
                    c = 2 * pair + par
                    xv = xts[pair][par * K:(par + 1) * K, :]
                    v2 = v2s[c]
                    cmat = eb[:, par * RS:(par + 1) * RS]
                    ht = hp.tile([H, CC], f32r, tag="ht", name=f"ht_{c}")
                    bps = [pp.tile([H, RS], f32, tag="ps", bufs=6,
                                   name=f"bps{c}_{i}") for i in range(8)]
                    for r in range(4, 8):
                        nc.tensor.matmul(
                            bps[r][:],
                            p4[:], v2[:, (r - 4) * RS:(r - 3) * RS],
                            start=True, stop=False)
                    for r in range(8):
                        nc.tensor.matmul(
                            bps[r][:],
                            lag[:, r * H:(r + 1) * H], cmat,
                            start=(r < 4), stop=True)
                    for r in range(8):
                        nc.vector.tensor_add(
                            ht[:, r * RS:(r + 1) * RS],
                            v2[:, r * RS:(r + 1) * RS], bps[r][:])
                    for piece in range(2):
                        sl = slice(piece * CC // 2, (piece + 1) * CC // 2)
                        nc.sync.dma_start(
                            hid[:, c * CC + piece * CC // 2:
                                c * CC + (piece + 1) * CC // 2],
                            ht[:, sl].bitcast(f32))

                    knt = knp.tile([2 * K, 4 * RS], f32, tag="knt",
                                   name=f"knt_{c}")
                    kps = [pp.tile([H, RS], f32, tag="ps", bufs=6,
                                   name=f"kps{c}_{i}") for i in range(8)]
                    a1v = a1[par * K:(par + 1) * K, :]
                    for r in range(8):
                        nc.tensor.matmul(kps[r][0:K, :], a1v,
                                         xv[:, r * RS:(r + 1) * RS],
                                         start=True, stop=False)
                    for r in range(8):
                        hpv = (ht[:, (r - 1) * RS:r * RS] if r >= 1
                               else cmat)
                        nc.tensor.matmul(kps[r][0:K, :], a2[:], hpv,
                                         start=False, stop=True)
                    for r in range(8):
                        # rows: r<4 -> knt[0:K] col-block r; r>=4 ->
                        # knt[K:2K] col-block r-4 (ACT/DVE place across
                        # partition halves; alternate engines to balance)
                        dst = (knt[0:K, r * RS:(r + 1) * RS] if r < 4 else
                               knt[K:2 * K, (r - 4) * RS:(r - 3) * RS])
                        nc.scalar.copy(dst, kps[r][0:K, :])
                    for piece in range(2):
                        nc.sync.dma_start(
                            kn[:, c * 4 * RS + piece * 2 * RS:
                                c * 4 * RS + (piece + 1) * 2 * RS],
                            knt[:, piece * 2 * RS:(piece + 1) * 2 * RS])

            # software-pipelined emission: scan(i) interleaved with conv(i+1)
            prev_last = h0
            emit_xt(0)
            for f in conv_pieces(0):
                f()
            for i in range(NP):
                if i + 1 < NP:
                    emit_xt(i + 1)
                    fillers = conv_pieces(i + 1)
                else:
                    fillers = iter(())
                eb = emit_scan(i, prev_last, fillers)
                prev_last = eb[:, 64 * BL:EC]
                emit_b(i, eb)

    nc.compile()
    return nc


def _get_program():
    if "nc" not in _cache:
        _cache["nc"] = _build_program()
    return _cache["nc"]


def _host_weights(A1, A2, A3, A4):
    A1d, A2d, A3d, A4d = (np.asarray(a, np.float64) for a in (A1, A2, A3, A4))
    pw = [np.eye(H)]
    for _ in range(8):
        pw.append(pw[-1] @ A4d)
    wconv = np.concatenate(
        [(pw[d] @ A3d).T for d in range(4)], axis=1).astype(np.float32)
    p4 = pw[4].T.astype(np.float32)
    lag = np.concatenate(
        [pw[r + 1].T for r in range(8)], axis=1).astype(np.float32)
    g8 = pw[8]
    qmats = []
    for e in E8:
        qmats.append(np.linalg.matrix_power(g8, e).T.astype(np.float32))
    qin = np.concatenate(qmats, axis=1)
    return (np.ascontiguousarray(wconv), np.ascontiguousarray(p4),
            np.ascontiguousarray(lag), np.ascontiguousarray(qin),
            np.ascontiguousarray(A1d.T.astype(np.float32)),
            np.ascontiguousarray(A2d.T.astype(np.float32)))


def _in_maps(x, h0, weights):
    wconv, p4, lag, qin, a1t, a2t = weights
    maps = []
    for core in range(NCORES):
        xc = x[:, :, core * BL:(core + 1) * BL]          # [T, K, BL]
        xr = xc.reshape(NCH, Q, M, K, BL)                # [c, q, r, k, b]
        xdev = np.ascontiguousarray(
            xr.transpose(3, 0, 2, 1, 4).reshape(K, NCOL))  # [k,(c,r,q,b)]
        maps.append({
            "xin": xdev,
            "h0in": np.ascontiguousarray(h0[:, core * BL:(core + 1) * BL]),
            "wconv": wconv, "p4in": p4, "lagin": lag, "qin": qin,
            "a1in": a1t, "a2in": a2t,
        })
    return maps


def kernel(x, init_hidden_states, A1, A2, A3, A4):
    from concourse.bass_utils import run_bass_kernel_spmd

    x = np.asarray(x, np.float32)
    h0 = np.asarray(init_hidden_states, np.float32)
    weights = _host_weights(A1, A2, A3, A4)

    nc = _get_program()
    results = run_bass_kernel_spmd(
        nc, _in_maps(x, h0, weights), core_ids=list(range(NCORES))).results

    known = np.empty((T, K, B), np.float32)
    hidden = np.empty((T, H, B), np.float32)
    for core in range(NCORES):
        bs = slice(core * BL, (core + 1) * BL)
        hd = results[core]["hid"].reshape(H, NCH, M, Q, BL)
        hidden[:, :, bs] = hd.transpose(1, 3, 2, 0, 4).reshape(T, H, BL)
        # kn rows: p<64 -> k=p, r=j; p>=64 -> k=p-64, r=j+4
        kd = results[core]["kn"].reshape(2, K, NCH, 4, Q, BL)
        kd2 = kd.transpose(2, 4, 0, 3, 1, 5)  # [c, q, half, j, k, b]
        kd2 = kd2.reshape(NCH, Q, M, K, BL)   # [c, q, r, k, b], r=half*4+j
        known[:, :, bs] = kd2.reshape(T, K, BL)
    return known, hidden


# revision 34
# speedup vs baseline: 1.0506x; 1.0079x over previous
"""Bass/Trainium2 kernel for nn_CustomLinearRNN.

Reference recurrence (T=4096, K=64, H=128, B=128):
    known_t = A1 @ x_t + A2 @ h_{t-1}
    h_t     = A3 @ x_t + A4 @ h_{t-1}
returns (known_seq [T,K,B], hidden_seq [T,H,B]).

Strategy: pure data-parallel over batch (B=128 -> 16 per core, 8 cores).
Per core the sequential scan is restructured into batched matmuls (fp32r):
  - time split into chunks of Tc=256; each chunk into Q=32 fine blocks of
    m=8 steps; columns laid out r-major (r = s%8) so every lagged matmul
    hits contiguous 512-column slices.
  - V2 = depth-4 causal conv of x with (A4^d A3), d=0..3 (masked at fine
    block boundaries), accumulated in PSUM.
  - fine level 3: V3[s] = V2[s] + A4^4 V2[s-4] (folded into broadcast).
  - block-end sums E_q = V3 at r=7; a seeded radix-4 Hillis-Steele scan
    over the 65-entry pair sequence [h_in, E_0..E_63] with powers of A4^8
    yields S_{q-1} = state entering each fine block + the pair-exit state.
  - broadcast: H[s] = V2[s] + (r>=4: A4^4 V2[s-4]) + A4^{r+1} S_{q-1}.
  - known[s] = A1 x[s] + A2 h[s-1]; h[s-1] = (r-1)-slice of H, or S_{q-1}
    for r=0; r and r+4 slices share one PSUM tile via partition packing.
All matrix powers/products precomputed on host in float64.
"""

import numpy as np

T, K, H, B = 4096, 64, 128, 128
NCORES = 8
BL = B // NCORES            # 16 batch per core
TC = 256                    # chunk length (timesteps)
NCH = T // TC               # 16 chunks
M = 8                       # fine block
Q = TC // M                 # 32 fine blocks per chunk
RS = Q * BL                 # 512 = columns per r-slice
CC = TC * BL                # 4096 = columns per chunk
NCOL = T * BL               # 65536 columns per core
NE = 2 * Q + 1              # 65 scan entries per chunk-pair
EC = NE * BL                # 1040 columns in the pair scan buffer
E8 = [1, 2, 3, 4, 8, 12, 16, 32, 48, 64]   # powers of A4^8 for the scan

_cache = {}


def _build_program():
    import concourse.tile as tile
    from concourse import bacc, mybir

    nc = bacc.Bacc("TRN2", target_bir_lowering=False, debug=False,
                   num_devices=NCORES)
    f32 = mybir.dt.float32
    f32r = mybir.dt.float32r

    xin = nc.dram_tensor("xin", [K, NCOL], f32r, kind="ExternalInput").ap()
    h0in = nc.dram_tensor("h0in", [H, BL], f32r, kind="ExternalInput").ap()
    wconv = nc.dram_tensor("wconv", [K, 4 * H], f32r,
                           kind="ExternalInput").ap()
    p4in = nc.dram_tensor("p4in", [H, H], f32r, kind="ExternalInput").ap()
    lagin = nc.dram_tensor("lagin", [H, 8 * H], f32r,
                           kind="ExternalInput").ap()
    qin = nc.dram_tensor("qin", [H, len(E8) * H], f32r,
                         kind="ExternalInput").ap()
    a1in = nc.dram_tensor("a1in", [K, K], f32r, kind="ExternalInput").ap()
    a2in = nc.dram_tensor("a2in", [H, K], f32r, kind="ExternalInput").ap()
    hid = nc.dram_tensor("hid", [H, NCOL], f32, kind="ExternalOutput").ap()
    kn = nc.dram_tensor("kn", [2 * K, NCOL // 2], f32,
                        kind="ExternalOutput").ap()

    with tile.TileContext(nc) as tc:
        from contextlib import ExitStack
        with ExitStack() as ctx:
            wp = ctx.enter_context(tc.tile_pool(name="weights", bufs=1))
            xp = ctx.enter_context(tc.tile_pool(name="xtiles", bufs=3))
            v2p = ctx.enter_context(tc.tile_pool(name="v2", bufs=4))
            hp = ctx.enter_context(tc.tile_pool(name="ht", bufs=3))
            knp = ctx.enter_context(tc.tile_pool(name="kn", bufs=2))
            ep = ctx.enter_context(tc.tile_pool(name="escan", bufs=2))
            pp = ctx.enter_context(
                tc.tile_pool(name="psum", bufs=3, space="PSUM"))

            w_all = wp.tile([2 * K, 4 * H], f32r, tag="w_all")
            nc.sync.dma_start(w_all[0:K, :], wconv)
            nc.sync.dma_start(w_all[K:2 * K, :], wconv)
            p4 = wp.tile([H, H], f32r, tag="p4")
            nc.sync.dma_start(p4[:], p4in)
            lag = wp.tile([H, 8 * H], f32r, tag="lag")
            nc.sync.dma_start(lag[:], lagin)
            qw = wp.tile([H, len(E8) * H], f32r, tag="qw")
            nc.sync.dma_start(qw[:], qin)
            a1 = wp.tile([2 * K, K], f32r, tag="a1")
            nc.sync.dma_start(a1[0:K, :], a1in)
            nc.sync.dma_start(a1[K:2 * K, :], a1in)
            a2 = wp.tile([H, K], f32r, tag="a2")
            nc.sync.dma_start(a2[:], a2in)
            h0 = wp.tile([H, BL], f32r, tag="h0")
            nc.sync.dma_start(h0[:], h0in)

            def G(e):  # lhsT of (A4^8)^e
                j = E8.index(e)
                return qw[:, j * H:(j + 1) * H]

            NP = NCH // 2
            xts = {}
            v2s = {}          # chunk index -> V2 tile

            def emit_xt(pair):
                xt = xp.tile([2 * K, CC], f32r, tag="xt",
                             name=f"xt_{pair}")
                xts[pair] = xt
                npc = 8 if pair == 0 else 2
                for par in range(2):
                    cbase = (2 * pair + par) * CC
                    for piece in range(npc):
                        sl = slice(piece * CC // npc,
                                   (piece + 1) * CC // npc)
                        nc.sync.dma_start(
                            xt[par * K:(par + 1) * K, sl],
                            xin[:, cbase + piece * CC // npc:
                                cbase + (piece + 1) * CC // npc])

            def conv_pieces(pair):
                """Yield conv emission pieces for both chunks of a pair."""
                for par in range(2):
                    c = 2 * pair + par
                    xv = xts[pair][par * K:(par + 1) * K, :]
                    v2 = v2p.tile([H, CC], f32r, tag="v2", name=f"v2_{c}")
                    v2s[c] = v2
                    cps = [pp.tile([H, RS], f32, tag="ps", bufs=6,
                                   name=f"cps{c}_{i}") for i in range(8)]

                    def piece(d, par=par, c=c, xv=xv, v2=v2, cps=cps):
                        wd = w_all[par * K:(par + 1) * K,
                                   d * H:(d + 1) * H]
                        for r in range(d, 8):
                            nc.tensor.matmul(
                                cps[r][:],
                                wd, xv[:, (r - d) * RS:(r - d + 1) * RS],
                                start=(d == 0), stop=(d == min(r, 3)))
                        lo, hi = (d, d + 1) if d < 3 else (3, 8)
                        for r in range(lo, hi):
                            nc.scalar.copy(v2[:, r * RS:(r + 1) * RS],
                                           cps[r][:])
                    yield from (lambda d=d: piece(d) for d in range(4))

            def emit_scan(pair, prev_last, fillers):
                """Seeded radix-4 scan; calls next(fillers) between rounds."""
                eb = ep.tile([H, EC], f32r, tag="eb", name=f"eb_{pair}")
                nc.vector.tensor_copy(eb[:, 0:BL], prev_last[:, 0:BL])
                for par in range(2):
                    xps = pp.tile([H, RS], f32, tag="eps", bufs=2,
                                  name=f"xps{pair}_{par}")
                    nc.tensor.matmul(xps[:], p4[:],
                                     v2s[2 * pair + par][:, 3 * RS:4 * RS],
                                     start=True, stop=True)
                    nc.vector.tensor_add(
                        eb[:, (1 + par * Q) * BL:(1 + (par + 1) * Q) * BL],
                        v2s[2 * pair + par][:, 7 * RS:8 * RS], xps[:])
                for base in (1, 4, 16):
                    for f in (next(fillers, None), next(fillers, None)):
                        if f is not None:
                            f()
                    a_lags = [v for v in (base, 2 * base, 3 * base)
                              if v * BL < RS + BL]
                    b_lags = [base, 2 * base, 3 * base]
                    if base == 16:
                        b_lags.append(64)
                    pa = pp.tile([H, RS], f32, tag="eps", bufs=2,
                                 name=f"pa{pair}_{base}")
                    pb = pp.tile([H, RS], f32, tag="eps", bufs=2,
                                 name=f"pb{pair}_{base}")
                    na = RS + BL - base * BL   # cols in part a
                    for i, v in enumerate(a_lags):
                        nc.tensor.matmul(
                            pa[:, (v - base) * BL:na],
                            G(v), eb[:, 0:RS + BL - v * BL],
                            start=(i == 0), stop=(i == len(a_lags) - 1))
                    for i, v in enumerate(b_lags):
                        lo = max(RS + BL, v * BL)
                        nc.tensor.matmul(
                            pb[:, lo - (RS + BL):RS],
                            G(v), eb[:, lo - v * BL:EC - v * BL],
                            start=(i == 0), stop=(i == len(b_lags) - 1))
                    nc.vector.tensor_add(eb[:, base * BL:RS + BL],
                                         eb[:, base * BL:RS + BL],
                                         pa[:, 0:na])
                    nc.vector.tensor_add(eb[:, RS + BL:EC],
                                         eb[:, RS + BL:EC], pb[:])
                for f in fillers:
                    f()
                return eb

            def emit_b(pair, eb):
                for par in range(2):
                    c = 2 * pair + par
                    xv = xts[pair][par * K:(par + 1) * K, :]
                    v2 = v2s[c]
                    cmat = eb[:, par * RS:(par + 1) * RS]
                    ht = hp.tile([H, CC], f32r, tag="ht", name=f"ht_{c}")
                    kps = [pp.tile([H, RS], f32, tag="ps", bufs=6,
                                   name=f"kps{c}_{i}") for i in range(8)]
                    a1v = a1[par * K:(par + 1) * K, :]
                    for r in range(2):
                        nc.tensor.matmul(kps[r][0:K, :], a1v,
                                         xv[:, r * RS:(r + 1) * RS],
                                         start=True, stop=False)
                    bps = [pp.tile([H, RS], f32, tag="ps", bufs=6,
                                   name=f"bps{c}_{i}") for i in range(8)]
                    for r in range(4, 8):
                        nc.tensor.matmul(
                            bps[r][:],
                            p4[:], v2[:, (r - 4) * RS:(r - 3) * RS],
                            start=True, stop=False)
                    for r in range(8):
                        nc.tensor.matmul(
                            bps[r][:],
                            lag[:, r * H:(r + 1) * H], cmat,
                            start=(r < 4), stop=True)
                    for r in range(8):
                        nc.vector.tensor_add(
                            ht[:, r * RS:(r + 1) * RS],
                            v2[:, r * RS:(r + 1) * RS], bps[r][:])
                    for piece in range(2):
                        sl = slice(piece * CC // 2, (piece + 1) * CC // 2)
                        nc.sync.dma_start(
                            hid[:, c * CC + piece * CC // 2:
                                c * CC + (piece + 1) * CC // 2],
                            ht[:, sl].bitcast(f32))

                    knt = knp.tile([2 * K, 4 * RS], f32, tag="knt",
                                   name=f"knt_{c}")
                    for r in range(2, 8):
                        nc.tensor.matmul(kps[r][0:K, :], a1v,
                                         xv[:, r * RS:(r + 1) * RS],
                                         start=True, stop=False)
                    for r in range(8):
                        hpv = (ht[:, (r - 1) * RS:r * RS] if r >= 1
                               else cmat)
                        nc.tensor.matmul(kps[r][0:K, :], a2[:], hpv,
                                         start=False, stop=True)
                    for r in range(8):
                        # rows: r<4 -> knt[0:K] col-block r; r>=4 ->
                        # knt[K:2K] col-block r-4 (ACT/DVE place across
                        # partition halves; alternate engines to balance)
                        dst = (knt[0:K, r * RS:(r + 1) * RS] if r < 4 else
                               knt[K:2 * K, (r - 4) * RS:(r - 3) * RS])
                        nc.scalar.copy(dst, kps[r][0:K, :])
                    for piece in range(2):
                        nc.sync.dma_start(
                            kn[:, c * 4 * RS + piece * 2 * RS:
                                c * 4 * RS + (piece + 1) * 2 * RS],
                            knt[:, piece * 2 * RS:(piece + 1) * 2 * RS])

            # software-pipelined emission: scan(i) interleaved with conv(i+1)
            prev_last = h0
            emit_xt(0)
            for f in conv_pieces(0):
                f()
            for i in range(NP):
                if i + 1 < NP:
                    emit_xt(i + 1)
                    fillers = conv_pieces(i + 1)
                else:
                    fillers = iter(())
                eb = emit_scan(i, prev_last, fillers)
                prev_last = eb[:, 64 * BL:EC]
                emit_b(i, eb)

    nc.compile()
    return nc


def _get_program():
    if "nc" not in _cache:
        _cache["nc"] = _build_program()
    return _cache["nc"]


def _host_weights(A1, A2, A3, A4):
    A1d, A2d, A3d, A4d = (np.asarray(a, np.float64) for a in (A1, A2, A3, A4))
    pw = [np.eye(H)]
    for _ in range(8):
        pw.append(pw[-1] @ A4d)
    wconv = np.concatenate(
        [(pw[d] @ A3d).T for d in range(4)], axis=1).astype(np.float32)
    p4 = pw[4].T.astype(np.float32)
    lag = np.concatenate(
        [pw[r + 1].T for r in range(8)], axis=1).astype(np.float32)
    g8 = pw[8]
    qmats = []
    for e in E8:
        qmats.append(np.linalg.matrix_power(g8, e).T.astype(np.float32))
    qin = np.concatenate(qmats, axis=1)
    return (np.ascontiguousarray(wconv), np.ascontiguousarray(p4),
            np.ascontiguousarray(lag), np.ascontiguousarray(qin),
            np.ascontiguousarray(A1d.T.astype(np.float32)),
            np.ascontiguousarray(A2d.T.astype(np.float32)))


def _in_maps(x, h0, weights):
    wconv, p4, lag, qin, a1t, a2t = weights
    maps = []
    for core in range(NCORES):
        xc = x[:, :, core * BL:(core + 1) * BL]          # [T, K, BL]
        xr = xc.reshape(NCH, Q, M, K, BL)                # [c, q, r, k, b]
        xdev = np.ascontiguousarray(
            xr.transpose(3, 0, 2, 1, 4).reshape(K, NCOL))  # [k,(c,r,q,b)]
        maps.append({
            "xin": xdev,
            "h0in": np.ascontiguousarray(h0[:, core * BL:(core + 1) * BL]),
            "wconv": wconv, "p4in": p4, "lagin": lag, "qin": qin,
            "a1in": a1t, "a2in": a2t,
        })
    return maps


def kernel(x, init_hidden_states, A1, A2, A3, A4):
    from concourse.bass_utils import run_bass_kernel_spmd

    x = np.asarray(x, np.float32)
    h0 = np.asarray(init_hidden_states, np.float32)
    weights = _host_weights(A1, A2, A3, A4)

    nc = _get_program()
    results = run_bass_kernel_spmd(
        nc, _in_maps(x, h0, weights), core_ids=list(range(NCORES))).results

    known = np.empty((T, K, B), np.float32)
    hidden = np.empty((T, H, B), np.float32)
    for core in range(NCORES):
        bs = slice(core * BL, (core + 1) * BL)
        hd = results[core]["hid"].reshape(H, NCH, M, Q, BL)
        hidden[:, :, bs] = hd.transpose(1, 3, 2, 0, 4).reshape(T, H, BL)
        # kn rows: p<64 -> k=p, r=j; p>=64 -> k=p-64, r=j+4
        kd = results[core]["kn"].reshape(2, K, NCH, 4, Q, BL)
        kd2 = kd.transpose(2, 4, 0, 3, 1, 5)  # [c, q, half, j, k, b]
        kd2 = kd2.reshape(NCH, Q, M, K, BL)   # [c, q, r, k, b], r=half*4+j
        known[:, :, bs] = kd2.reshape(T, K, BL)
    return known, hidden
